# revision 22
# baseline (speedup 1.0000x reference)
"""Trainium2 Bass kernel for nn_DeformableConvLSTMCell_33895881900284.

Full (unsharded) inputs in, full outputs out. Internally: data-parallel over
batch across 8 NeuronCores (8 batches per core), conv weights / gate params
replicated.

Math per the reference:
  outI  = conv3x3_same(inputs, wconvInput)
  g     = tanh(outI + conv3x3_same(hidden_prev, wconvHidden) + gateBias)
  gapI  = mean_hw(outI);  gapH = mean_hw(hidden_prev)          # [B, D]
  i/f/o = sigmoid(wx*gapI + wh*gapH + bias)                    # [B, D]
  tiled gate: value used at (b, h, w, c) is gate[(28*b + h) % 64, c]
  state  = f*state_prev + i*g;  hidden = o*tanh(state)

The (28*b+h)%64 scrambling makes gates cross-batch: each core computes its
local GAP columns, all cores AllGather them, and a per-core index-array input
drives an indirect-DMA gather of exactly the gate rows this core's outputs
need (the SPMD program stays identical across cores; only input data differs).

gapI never touches the conv output: by linearity, 784*gapI is a combination
of 9 masked pixel sums of the raw input (full sum, edge rows/cols, corners)
matmul'd with summed conv-weight taps ("stage A"). Groups 1-4 carry negative
coefficients; the sign is folded into the raw sums so the final combine is a
pure PSUM accumulation over individual weight taps (no combined-A tile).

Pipeline design (PE is the bottleneck engine, keep it saturated):
  - one big 3D DMA per (batch, tensor): [112, 7, 256]; stage-A masked-sum
    matmuls ride the same natural tiles for batches 0-2; batches 3-7 get
    dedicated paced stage-A loads so the AllGather fires ~1/3 into the run.
  - all PE transposes use a bf16 identity as the moving operand (1.0
    cycles/row instead of 2.0 for f32, exact) on f32r-bitcast data.
  - conv = 36 shifted matmuls per (window, dc): stationary weights in bf16,
    moving activations f32r (full rate either way); gateBias is added by DVE
    from PSUM (no PE identity-matmul), ACT applies tanh into bf16 g-tiles.
  - the gate gather/transpose block sits after batch-3's convs in the PE
    stream, so the collective's fixed latency hides under conv work;
    elementwise for batches 0-3 runs right after the gates, batches 4-7
    inline. Outputs are stored per conv window to shrink the tail.
Outputs leave the chip transposed ([dc, 128, pix]); the host reassembles.
"""
import numpy as np

import bass_rust
import concourse.bass as bass
import concourse.mybir as mybir
import concourse.tile as tile
from concourse.bass_utils import run_bass_kernel_spmd

F32 = mybir.dt.float32
F32R = mybir.dt.float32r
BF16 = mybir.dt.bfloat16
I32 = mybir.dt.int32
AF = mybir.ActivationFunctionType
ALU = mybir.AluOpType

N_CORES = 8
B, H, W, CIN, D = 64, 28, 28, 256, 256
BL = B // N_CORES          # local batches per core
PIX = H * W                # 784
PG = 112                   # pixels per transpose group (4 rows)
NPG = PIX // PG            # 7
PAD = 30                   # padded row/col length
XTLEN = PAD * PAD          # 900
NW = 2                     # windows per batch
WROWS = H // NW            # 14
WN = WROWS * W             # 392
NCC = CIN // 128           # 2 channel chunks
NDC = D // 128             # 2 output-channel chunks

# tap order t = 3*kh + kw ; dh = kh-1, dw = kw-1
TAPS = [(kh, kw) for kh in range(3) for kw in range(3)]

# gapI tap expansion: (mask group, taps). Groups 1-4 are negative; the sign
# lives in the raw sums (negated at stage-A finalize).
GAP_TERMS = [(0, list(range(9))), (1, [6, 7, 8]), (2, [0, 1, 2]),
             (3, [2, 5, 8]), (4, [0, 3, 6]),
             (5, [8]), (6, [6]), (7, [2]), (8, [0])]

# ---------------------------------------------------------------------------
# walrus fixup: split semaphore waits that exceed the per-instruction budget
# (observed: Drain and Matmult accept only 1 semaphore wait each).
MAX_WAITS = 1


def _split_excess_sem_waits(nc):
    counter = [0]
    for fn in nc.m.functions:
        for bb in fn.blocks:
            insts = bb.instructions
            i = 0
            while i < len(insts):
                inst = insts[i]
                si = inst.sync_info
                if si is not None and si.on_wait and len(si.on_wait) > MAX_WAITS:
                    waits = list(si.on_wait)
                    excess = waits[:-MAX_WAITS]
                    keep = waits[-MAX_WAITS:]
                    new_insts = []
                    for j in range(0, len(excess), MAX_WAITS):
                        chunk = excess[j:j + MAX_WAITS]
                        noop = mybir.InstNoOp(
                            name=f"I-waitsplit-{counter[0]}", ins=[], outs=[])
                        counter[0] += 1
                        noop.engine = inst.engine
                        noop.sync_info = bass_rust.SyncInfo(
                            on_wait=chunk, on_update=[])
                        nc.register_instruction(noop)
                        new_insts.append(noop)
                    inst.sync_info = bass_rust.SyncInfo(
                        on_wait=keep, on_update=list(si.on_update))
                    insts[i:i] = new_insts
                    i += len(new_insts)
                i += 1
    return nc


# ---------------------------------------------------------------------------
def _r3(ap, w):
    """view flat free dim as (rows, w)"""
    return ap.rearrange("c (r w) -> c r w", w=w)


def build_nc():
    nc = bass.Bass("TRN2", target_bir_lowering=False, debug=False,
                   num_devices=N_CORES)

    dram = {}
    dram["inputs"] = nc.dram_tensor("inputs", [BL, PIX, CIN], BF16,
                                    kind="ExternalInput").ap()
    dram["state_prev"] = nc.dram_tensor("state_prev", [BL, PIX, D], F32,
                                        kind="ExternalInput").ap()
    dram["hidden_prev"] = nc.dram_tensor("hidden_prev", [BL, PIX, D], BF16,
                                         kind="ExternalInput").ap()
    dram["wpk"] = nc.dram_tensor("wconv_bf16", [128, 2 * 9 * NCC * 256],
                                 BF16, kind="ExternalInput").ap()
    dram["gb"] = nc.dram_tensor("gateBias", [PIX, D], F32,
                                kind="ExternalInput").ap()
    dram["vec"] = {}
    for nm in ("wxi", "whi", "inputBias", "wxf", "whf", "forgetBias",
               "wxo", "who", "outputBias"):
        dram["vec"][nm] = nc.dram_tensor(nm, [D, 1], F32,
                                         kind="ExternalInput").ap()
    dram["ident"] = nc.dram_tensor("identity", [128, 128], F32,
                                   kind="ExternalInput").ap()
    dram["idx"] = nc.dram_tensor("gate_idx", [4, 224, 1], I32,
                                 kind="ExternalInput").ap()
    dram["masks"] = nc.dram_tensor("gap_masks", [NPG, PG, 9], BF16,
                                   kind="ExternalInput").ap()
    dram["hidden"] = nc.dram_tensor("hidden", [BL, NDC, 128, PIX], F32,
                                    kind="ExternalOutput").ap()
    dram["state"] = nc.dram_tensor("state", [BL, NDC, 128, PIX], F32,
                                   kind="ExternalOutput").ap()
    dram["cc_in"] = nc.dram_tensor("cc_in", [32, 128], F32, kind="Internal").ap()
    dram["cc_out"] = nc.dram_tensor("cc_out", [N_CORES * 32, 128], F32,
                                    kind="Internal", addr_space="Shared").ap()

    ctx_mgr = nc.allow_low_precision("bf16 weights + float32r operands for PE")
    ctx_mgr.__enter__()
    with tile.TileContext(nc) as tc:
        _build_body(nc, tc, dram)
    ctx_mgr.__exit__(None, None, None)
    return nc


def _build_body(nc, tc, dram):
    from collections import deque
    from contextlib import ExitStack
    ctx = ExitStack()
    pool = lambda **kw: ctx.enter_context(tc.tile_pool(**kw))

    const = pool(name="const", bufs=1)
    wts = pool(name="wts", bufs=1)
    wst = pool(name="wst", bufs=1)         # f32 weight / gateBias staging
    natb = pool(name="natb", bufs=3)       # stage-B natural loads (in/hid)
    nata = pool(name="nata", bufs=2)       # stage-A-only loads (j=3..7)
    natsp = pool(name="natsp", bufs=4)     # state_prev natural loads
    xt_in = pool(name="xt_in", bufs=2)
    xt_hid = pool(name="xt_hid", bufs=2)
    xt_sp = pool(name="xt_sp", bufs=5)
    ew = pool(name="ew", bufs=2)
    gtp = pool(name="gtp", bufs=20)        # bf16 g tiles (deferred ew b0-3)
    outb = pool(name="outb", bufs=3)
    gsm = pool(name="gsm", bufs=1)
    gtmp = pool(name="gtmp", bufs=2)
    ps_conv = pool(name="ps_conv", bufs=3, space="PSUM")
    ps_tr = pool(name="ps_tr", bufs=2, space="PSUM")
    ps_gap = pool(name="ps_gap", bufs=1, space="PSUM")

    # ---- constants (deadline-ordered: b0 data + identity first) ----
    idx_sb = []
    vecs = {}

    def load_idx_and_vecs():
        for g4 in range(4):
            halves = []
            for hf in range(2):
                t = const.tile([PG, 1], I32, tag=f"idx{g4}_{hf}")
                nc.sync.dma_start(t[:],
                                  dram["idx"][g4, hf * PG:(hf + 1) * PG, :])
                halves.append(t)
            idx_sb.append(halves)
        for nm in dram["vec"]:
            t = const.tile([128, NDC], F32, tag=f"vec_{nm}")
            for c in range(NDC):
                nc.scalar.dma_start(t[:, c:c + 1],
                                    dram["vec"][nm][c * 128:(c + 1) * 128, :])
            if nm.startswith("wx") or nm.startswith("wh"):
                nc.vector.tensor_scalar_mul(t[:], t[:], 1.0 / PIX)
            vecs[nm] = t

    # ---- conv weights: f32 staged -> bf16 resident ----
    wconv = wts.tile([128, 2 * 9 * NCC * 256], BF16, tag="wconv")

    def wblk(conv, t, cc):
        off = ((conv * 9 + t) * NCC + cc) * 256
        return wconv[:, off:off + 256]

    def load_weights():
        half = 9 * NCC * 256
        nc.scalar.dma_start(wconv[:, 0:half], dram["wpk"][:, 0:half])
        nc.scalar.dma_start(wconv[:, half:2 * half],
                            dram["wpk"][:, half:2 * half])

    # gateBias: one big load, transposed to [128 d, PIX] bf16 per dc
    gbias = [const.tile([128, PIX], BF16, tag=f"gbias{dc}", name=f"gbias{dc}")
             for dc in range(NDC)]

    # stage-A accumulators
    raw = [gsm.tile([128, 9 * BL], BF16, tag=f"raw{cc}", name=f"raw{cc}")
           for cc in range(NCC)]
    gapH = [gsm.tile([128, BL], F32, tag=f"gapH{cc}", name=f"gapHs{cc}")
            for cc in range(NCC)]

    # ---- natural loads -----------------------------------------------------
    natI, natH, natS = {}, {}, {}     # stage-B tiles
    aI, aH = {}, {}                   # stage-A-only tiles (j=3..7)

    def load_nat(j, which):
        """one 3D DMA: [112, 7, 256] natural-layout tile"""
        if which == "in":
            src, p, tag = dram["inputs"], natb, "natb"
        elif which == "hid":
            src, p, tag = dram["hidden_prev"], natb, "natb"
        elif which == "ain":
            src, p, tag = dram["inputs"], nata, "nata"
        elif which == "ahid":
            src, p, tag = dram["hidden_prev"], nata, "nata"
        else:
            src, p, tag = dram["state_prev"], natsp, "natsp"
        dt_ = F32 if which == "sp" else BF16
        t = p.tile([PG, NPG, 256], dt_, tag=tag, name=f"nat_{which}_{j}")
        q = nc.scalar if which == "hid" else nc.sync
        q.dma_start(t[:], src[j].rearrange("(g p) d -> p g d", p=PG))
        return t

    # ---- stage A: masked pixel sums off a natural tile ---------------------
    def emit_stage_a(j, nat, is_input):
        p9 = ps_gap.tile([9, 256], F32, tag="rawT")
        for pg in range(NPG):
            nc.tensor.matmul(p9[:], masks[pg][:],
                             nat[:, pg, :],
                             start=(pg == 0), stop=(pg == NPG - 1))
        rt = gtmp.tile([9, 256], F32, tag="rawT_sb", bufs=1)
        nc.vector.tensor_copy(rt[:], p9[:])
        for cc in range(NCC):
            cs = slice(cc * 128, (cc + 1) * 128)
            pt = ps_tr.tile([128, 9], F32, tag="ptr9", bufs=1)
            nc.tensor.transpose(pt[:], rt[:, cs], ident[0:9, 0:9])
            if is_input:
                r3 = _r3(raw[cc][:], BL)
                nc.vector.tensor_copy(
                    r3[:, :, j:j + 1],
                    pt[:].rearrange("c (n o) -> c n o", o=1))
                # fold the negative sign of mask groups 1-4 into raw
                nc.vector.tensor_scalar_mul(r3[:, 1:5, j:j + 1],
                                            r3[:, 1:5, j:j + 1], -1.0)
            else:
                nc.vector.tensor_copy(gapH[cc][:, j:j + 1], pt[:, 0:1])

    # ---- stage B: transposes into padded conv layout -----------------------
    xin, xhid, xsp = {}, {}, {}

    def alloc_xt(j, which):
        if which == "in":
            p, dst, tag = xt_in, xin, "xin"
        elif which == "hid":
            p, dst, tag = xt_hid, xhid, "xhid"
        else:
            p, dst, tag = xt_sp, xsp, "xsp"
        tiles = []
        for cc in range(NCC):
            xlen = PIX if which == "sp" else XTLEN
            dt_ = F32 if which == "sp" else BF16
            xt = p.tile([128, xlen], dt_, tag=f"{tag}{cc}",
                        name=f"{tag}{cc}_{j}")
            if which != "sp":
                x3 = _r3(xt[:], PAD)
                nc.gpsimd.memset(x3[:, 0:1, :], 0.0)
                nc.gpsimd.memset(x3[:, PAD - 1:PAD, :], 0.0)
                nc.gpsimd.memset(x3[:, 1:PAD - 1, 0:1], 0.0)
                nc.gpsimd.memset(x3[:, 1:PAD - 1, PAD - 1:PAD], 0.0)
            tiles.append(xt)
        dst[j] = tiles
        return tiles

    def emit_transpose_group(nat, tiles, g7, padded):
        for cc in range(NCC):
            if padded:
                pt = ps_tr.tile([128, PG], BF16, tag="ptr")
                nc.tensor.transpose(pt[:],
                                    nat[:, g7, cc * 128:(cc + 1) * 128],
                                    identb[0:PG, 0:PG])
                dst = _r3(tiles[cc][:], PAD)[:, 1 + 4 * g7:1 + 4 * g7 + 4,
                                             1:29]
                nc.vector.tensor_copy(dst,
                                      pt[:].rearrange("c (r w) -> c r w", w=W))
            else:
                pt = ps_tr.tile([128, PG], F32, tag="ptr")
                nc.tensor.transpose(pt[:],
                                    nat[:, g7, cc * 128:(cc + 1) * 128],
                                    ident[0:PG, 0:PG])
                dst = _r3(tiles[cc][:, g7 * PG:(g7 + 1) * PG], W)
                nc.vector.tensor_copy(dst,
                                      pt[:].rearrange("c (r w) -> c r w", w=W))

    # ---- conv window: 36 matmuls + DVE bias add + ACT tanh -----------------
    def emit_conv_window(j, wi, dc):
        h0 = 1 + wi * WROWS
        base = (h0 - 1) * W
        p = ps_conv.tile([128, WN], F32, tag="pconv", name="pconv")
        p3 = _r3(p[:], W)
        first = True
        for conv, xbuf in ((0, xin[j]), (1, xhid[j])):
            for t, (kh, kw) in enumerate(TAPS):
                dh, dwid = kh - 1, kw - 1
                for cc in range(NCC):
                    rhs = _r3(xbuf[cc][:], PAD)[
                        :, h0 + dh:h0 + dh + WROWS, 1 + dwid:1 + dwid + W]
                    last = (conv == 1 and t == 8 and cc == NCC - 1)
                    nc.tensor.matmul(
                        p3, wblk(conv, t, cc)[:, dc * 128:(dc + 1) * 128],
                        rhs, start=first, stop=last)
                    first = False
        pre = ew.tile([128, WN], F32, tag="pre", bufs=3, name="pre")
        nc.vector.tensor_tensor(out=pre[:], in0=p[:],
                                in1=gbias[dc][:, base:base + WN], op=ALU.add)
        gt = gtp.tile([128, WN], BF16, tag="gt", name=f"gt{j}_{wi}_{dc}")
        nc.scalar.activation(gt[:], pre[:], AF.Tanh)
        return gt

    # ---- elementwise + per-window store ------------------------------------
    gates = {}

    def emit_elementwise(j, wi, dc, gt):
        h0 = 1 + wi * WROWS
        base = (h0 - 1) * W
        t0 = j * H + (h0 - 1)

        def gw(gate):
            return gates[gate][dc][:, t0:t0 + WROWS].to_broadcast(
                [128, WROWS, W])

        sp3 = _r3(xsp[j][dc][:, base:base + WN], W)
        g3 = _r3(gt[:], W)
        s1 = ew.tile([128, WN], F32, tag="s1", name="s1")
        nc.gpsimd.tensor_tensor(out=_r3(s1[:], W), in0=sp3, in1=gw("f"),
                                op=ALU.mult)
        s2 = ew.tile([128, WN], F32, tag="s2", name="s2")
        nc.gpsimd.tensor_tensor(out=_r3(s2[:], W), in0=g3, in1=gw("i"),
                                op=ALU.mult)
        st = outb.tile([128, WN], F32, tag="st", name=f"st{j}_{wi}_{dc}")
        nc.gpsimd.tensor_tensor(out=_r3(st[:], W), in0=_r3(s1[:], W),
                                in1=_r3(s2[:], W), op=ALU.add)
        th = ew.tile([128, WN], F32, tag="th", name="th")
        nc.scalar.activation(th[:], st[:], AF.Tanh)
        hd = outb.tile([128, WN], F32, tag="hd", name=f"hd{j}_{wi}_{dc}")
        nc.gpsimd.tensor_tensor(out=_r3(hd[:], W), in0=_r3(th[:], W),
                                in1=gw("o"), op=ALU.mult)
        nc.sync.dma_start(dram["state"][j, dc][:, base:base + WN], st[:])
        nc.scalar.dma_start(dram["hidden"][j, dc][:, base:base + WN], hd[:])

    # ---- PE-side work queue (drained between conv windows) -----------------
    peq = deque()

    def drain(n):
        for _ in range(min(n, len(peq))):
            peq.popleft()()

    def queue_batch_transposes(j):
        ti = alloc_xt(j, "in")
        th_ = alloc_xt(j, "hid")
        groups = []
        for g7 in range(NPG):
            groups.append(lambda g7=g7, j=j, ti=ti: emit_transpose_group(
                natI[j], ti, g7, True))
            groups.append(lambda g7=g7, j=j, th_=th_: emit_transpose_group(
                natH[j], th_, g7, True))
        if j == 0:
            groups = groups[0::2] + groups[1::2]
        peq.extend(groups)

    def queue_sp_transposes(j):
        ts = alloc_xt(j, "sp")
        for g7 in range(NPG):
            peq.append(lambda g7=g7, j=j, ts=ts: emit_transpose_group(
                natS[j], ts, g7, False))

    def queue_stage_a(j):
        if j <= 2:
            peq.append(lambda j=j: emit_stage_a(j, natI[j], True))
            peq.append(lambda j=j: emit_stage_a(j, natH[j], False))
        else:
            peq.append(lambda j=j: emit_stage_a(j, aI[j], True))
            peq.append(lambda j=j: emit_stage_a(j, aH[j], False))

    # ---- gap combine + collective ------------------------------------------
    def emit_gap_combine_and_allgather():
        gap_ps = ps_gap.tile([8, 256], F32, tag="gapI")
        terms = [(g, t, cc) for g, taps in GAP_TERMS for t in taps
                 for cc in range(NCC)]
        for i, (g, t, cc) in enumerate(terms):
            nc.tensor.matmul(gap_ps[:], raw[cc][:, g * BL:(g + 1) * BL],
                             wblk(0, t, cc),
                             start=(i == 0), stop=(i == len(terms) - 1))
        gapI_sb = gsm.tile([8, 256], F32, tag="gapI_sb")
        nc.vector.tensor_copy(gapI_sb[:], gap_ps[:])
        nc.sync.dma_start(dram["cc_in"][0:8, :], gapI_sb[:, 0:128])
        nc.sync.dma_start(dram["cc_in"][8:16, :], gapI_sb[:, 128:256])
        for cc in range(NCC):
            pt = ps_gap.tile([8, 128], F32, tag="gapI")
            nc.tensor.transpose(pt[:], gapH[cc][:], ident[:])
            hs = gsm.tile([8, 128], F32, tag=f"gapH_sb{cc}",
                          name=f"gapHsb{cc}")
            nc.vector.tensor_copy(hs[:], pt[:])
            nc.sync.dma_start(dram["cc_in"][16 + 8 * cc:24 + 8 * cc, :], hs[:])
        nc.gpsimd.collective_compute(
            "AllGather", ALU.bypass, replica_groups=[list(range(N_CORES))],
            ins=[dram["cc_in"][:]], outs=[dram["cc_out"][:]])

    def emit_gather_and_gates():
        sel = [gsm.tile([128, 224], BF16, tag=f"sel{g4}", name=f"sel{g4}")
               for g4 in range(4)]
        for g4 in range(4):
            for hf in range(2):
                gtile = gtmp.tile([PG, 128], F32, tag="gath", name="gath")
                nc.gpsimd.indirect_dma_start(
                    out=gtile[:], out_offset=None, in_=dram["cc_out"][:],
                    in_offset=bass.IndirectOffsetOnAxis(
                        ap=idx_sb[g4][hf][:, :1], axis=0))
                pt = ps_tr.tile([128, PG], F32, tag="ptr", name="pt_gath")
                nc.tensor.transpose(pt[:], gtile[:],
                                    ident[0:PG, 0:PG])
                nc.vector.tensor_copy(sel[g4][:, hf * PG:(hf + 1) * PG],
                                      pt[:])
        for gate, wx, wh, bi in (("i", "wxi", "whi", "inputBias"),
                                 ("f", "wxf", "whf", "forgetBias"),
                                 ("o", "wxo", "who", "outputBias")):
            per_dc = []
            for dc in range(NDC):
                t1 = gtmp.tile([128, 224], F32, tag="gm1", bufs=1, name="gm1")
                nc.vector.tensor_scalar_mul(t1[:], sel[dc][:],
                                            vecs[wx][:, dc:dc + 1])
                t2 = gtmp.tile([128, 224], F32, tag="gm2", bufs=1, name="gm2")
                nc.vector.tensor_scalar_mul(t2[:], sel[2 + dc][:],
                                            vecs[wh][:, dc:dc + 1])
                nc.vector.tensor_tensor(out=t1[:], in0=t1[:], in1=t2[:],
                                        op=ALU.add)
                gtb = gsm.tile([128, 224], F32, tag=f"gate_{gate}{dc}",
                               name=f"gate_{gate}{dc}")
                nc.scalar.activation(gtb[:], t1[:], AF.Sigmoid,
                                     bias=vecs[bi][:, dc:dc + 1])
                per_dc.append(gtb)
            gates[gate] = per_dc

    # ======================= emission schedule ==============================
    # b0 stage-B loads + identity first on the sync queue; weights first on
    # the act queue. Stage-A-only loads are paced in the slot loop (their DMA
    # dispatch can block the sync queue on buffer-free waits, so nothing
    # deadline-critical may follow them in the same slot).
    ident = const.tile([128, 128], F32, tag="ident")
    nc.sync.dma_start(ident[:], dram["ident"][:])
    identb = const.tile([128, 128], BF16, tag="identb")
    nc.vector.tensor_copy(identb[:], ident[:])

    natI[0] = load_nat(0, "in")
    natH[0] = load_nat(0, "hid")
    load_weights()
    masks = []
    for pg in range(NPG):
        m = const.tile([PG, 9], BF16, tag=f"mask{pg}", name=f"mask{pg}")
        nc.sync.dma_start(m[:], dram["masks"][pg])
        masks.append(m)
    natI[1] = load_nat(1, "in")
    natH[1] = load_nat(1, "hid")
    gnat = wst.tile([PG, NPG, 256], F32, tag="gnat", bufs=1)
    nc.sync.dma_start(gnat[:],
                      dram["gb"].rearrange("(g p) d -> p g d", p=PG))
    load_idx_and_vecs()

    def emit_gbias_group(g7):
        for dc in range(NDC):
            pt = ps_tr.tile([128, PG], F32, tag="ptr")
            nc.tensor.transpose(
                pt[:], gnat[:, g7, dc * 128:(dc + 1) * 128],
                ident[0:PG, 0:PG])
            nc.vector.tensor_copy(gbias[dc][:, g7 * PG:(g7 + 1) * PG],
                                  pt[:])

    # prologue PE work, drained now: batch-0 transposes + stage A for b0 +
    # gbias transposes (the first conv window's bias-add reads gbias, so it
    # must be written before slot 0 in DVE program order).
    queue_batch_transposes(0)
    queue_stage_a(0)
    for g7 in range(NPG):
        peq.append(lambda g7=g7: emit_gbias_group(g7))
    drain(len(peq))
    queue_batch_transposes(1)
    queue_stage_a(1)

    gts = {}
    for k in range(BL):
        # just-in-time loads (stage-B two batches ahead; stage-A paced)
        if k + 2 < BL:
            natI[k + 2] = load_nat(k + 2, "in")
            natH[k + 2] = load_nat(k + 2, "hid")
        if k == 0:
            for j in (3, 4):
                aI[j] = load_nat(j, "ain")
                aH[j] = load_nat(j, "ahid")
        elif k == 2:
            for j in (5, 6, 7):
                aI[j] = load_nat(j, "ain")
                aH[j] = load_nat(j, "ahid")
            for j in range(0, 4):
                natS[j] = load_nat(j, "sp")
        elif k >= 3 and k + 1 < BL:
            natS[k + 1] = load_nat(k + 1, "sp")

        # queue PE side work for this slot (b1 already queued in prologue)
        if 1 <= k and k + 1 < BL:
            queue_batch_transposes(k + 1)
        if k == 0:
            queue_stage_a(2)
        elif k == 1:
            queue_stage_a(3)
            queue_stage_a(4)
        elif k == 2:
            queue_stage_a(5)
            queue_stage_a(6)
            queue_stage_a(7)
        if k >= 5 and k + 1 < BL:
            queue_sp_transposes(k + 1)

        nwin = NW * NDC
        for wi in range(NW):
            for dc in range(NDC):
                gt = emit_conv_window(k, wi, dc)
                rem = len(peq)
                left = nwin - (wi * NDC + dc)
                drain(max(5, (rem + left - 1) // left))
                # ew is emitted after the drain so its sp-layout transposes
                # (front of the queue at k>=5) precede it in DVE order
                if k <= 4:
                    gts[(k, wi, dc)] = gt
                else:
                    emit_elementwise(k, wi, dc, gt)

        if k == 2:
            drain(len(peq))           # all stage A must be emitted
            emit_gap_combine_and_allgather()
        if k == 4:
            drain(len(peq))
            emit_gather_and_gates()
            for j in range(0, 4):
                queue_sp_transposes(j)
            drain(len(peq))
            for j in range(0, 2):
                for wi in range(NW):
                    for dc in range(NDC):
                        emit_elementwise(j, wi, dc, gts.pop((j, wi, dc)))
            queue_sp_transposes(4)
            drain(len(peq))
            for j in range(2, 5):
                for wi in range(NW):
                    for dc in range(NDC):
                        emit_elementwise(j, wi, dc, gts.pop((j, wi, dc)))
            queue_sp_transposes(5)
    drain(len(peq))

    ctx.close()


# ---------------------------------------------------------------------------
_NC_CACHE = None


def _get_nc():
    global _NC_CACHE
    if _NC_CACHE is None:
        nc = build_nc()
        _split_excess_sem_waits(nc)
        _NC_CACHE = nc
    return _NC_CACHE


def _gate_idx(core):
    idx = np.empty((4, 224, 1), np.int32)
    for j in range(BL):
        for hh in range(H):
            t = j * H + hh
            sel_b = (H * (BL * core + j) + hh) % B
            cp, bp = sel_b // BL, sel_b % BL
            for g in range(4):
                idx[g, t, 0] = cp * 32 + g * 8 + bp
    return idx


def _gap_masks():
    m = np.zeros((PIX, 9), np.float32)
    hw = np.arange(PIX)
    r, c = hw // W, hw % W
    m[:, 0] = 1.0
    m[r == 0, 1] = 1.0
    m[r == H - 1, 2] = 1.0
    m[c == 0, 3] = 1.0
    m[c == W - 1, 4] = 1.0
    m[(r == 0) & (c == 0), 5] = 1.0
    m[(r == 0) & (c == W - 1), 6] = 1.0
    m[(r == H - 1) & (c == 0), 7] = 1.0
    m[(r == H - 1) & (c == W - 1), 8] = 1.0
    return m.reshape(NPG, PG, 9)


def _make_in_maps(inputs):
    f32 = np.float32
    import ml_dtypes
    wpk = np.empty((128, 2 * 9 * NCC * 256), dtype=ml_dtypes.bfloat16)
    for conv, w in ((0, np.asarray(inputs["wconvInput"], dtype=f32)),
                    (1, np.asarray(inputs["wconvHidden"], dtype=f32))):
        for t, (kh, kw) in enumerate(TAPS):
            for cc in range(NCC):
                off = ((conv * 9 + t) * NCC + cc) * 256
                wpk[:, off:off + 256] = w[kh, kw,
                                          cc * 128:(cc + 1) * 128, :]
    shared = {
        "wconv_bf16": wpk,
        "gateBias": np.ascontiguousarray(inputs["gateBias"],
                                         dtype=f32).reshape(PIX, D),
        "identity": np.eye(128, dtype=f32),
        "gap_masks": _gap_masks().astype(ml_dtypes.bfloat16),
    }
    for nm in ("wxi", "whi", "inputBias", "wxf", "whf", "forgetBias",
               "wxo", "who", "outputBias"):
        shared[nm] = np.ascontiguousarray(inputs[nm], dtype=f32).reshape(D, 1)

    bf16 = ml_dtypes.bfloat16
    xin = np.ascontiguousarray(inputs["inputs"], dtype=f32).reshape(
        B, PIX, CIN).astype(bf16)
    xsp = np.ascontiguousarray(inputs["state_prev"], dtype=f32).reshape(B, PIX, D)
    xhp = np.ascontiguousarray(inputs["hidden_prev"], dtype=f32).reshape(
        B, PIX, D).astype(bf16)

    in_maps = []
    for k in range(N_CORES):
        sl = slice(k * BL, (k + 1) * BL)
        m = dict(shared)
        m["inputs"] = xin[sl]
        m["state_prev"] = xsp[sl]
        m["hidden_prev"] = xhp[sl]
        m["gate_idx"] = _gate_idx(k)
        in_maps.append(m)
    return in_maps


def kernel(**inputs):
    nc = _get_nc()
    in_maps = _make_in_maps(inputs)
    res = run_bass_kernel_spmd(nc, in_maps, core_ids=list(range(N_CORES)))

    def unshard(name):
        # per-core outputs are [BL, NDC, 128, PIX] (channel-major); restore NHWC
        full = np.concatenate([res.results[k][name] for k in range(N_CORES)],
                              axis=0)
        return np.ascontiguousarray(full.transpose(0, 3, 1, 2)).reshape(
            B, H, W, D)

    return unshard("hidden"), unshard("state")


# revision 27
# speedup vs baseline: 1.0529x; 1.0529x over previous
"""Trainium2 Bass kernel for nn_DeformableConvLSTMCell_33895881900284.

Full (unsharded) inputs in, full outputs out. Internally: data-parallel over
batch across 8 NeuronCores (8 batches per core), conv weights / gate params
replicated.

Math per the reference:
  outI  = conv3x3_same(inputs, wconvInput)
  g     = tanh(outI + conv3x3_same(hidden_prev, wconvHidden) + gateBias)
  gapI  = mean_hw(outI);  gapH = mean_hw(hidden_prev)          # [B, D]
  i/f/o = sigmoid(wx*gapI + wh*gapH + bias)                    # [B, D]
  tiled gate: value used at (b, h, w, c) is gate[(28*b + h) % 64, c]
  state  = f*state_prev + i*g;  hidden = o*tanh(state)

The (28*b+h)%64 scrambling makes gates cross-batch: each core computes its
local GAP columns, all cores AllGather them, and a per-core index-array input
drives an indirect-DMA gather of exactly the gate rows this core's outputs
need (the SPMD program stays identical across cores; only input data differs).

gapI never touches the conv output: by linearity, 784*gapI is a combination
of 9 masked pixel sums of the raw input (full sum, edge rows/cols, corners)
matmul'd with summed conv-weight taps ("stage A"). Groups 1-4 carry negative
coefficients; the sign is folded into the raw sums so the final combine is a
pure PSUM accumulation over individual weight taps (no combined-A tile).

Pipeline design (PE is the bottleneck engine, keep it saturated):
  - one big 3D DMA per (batch, tensor): [112, 7, 256]; stage-A masked-sum
    matmuls ride the same natural tiles for batches 0-2; batches 3-7 get
    dedicated paced stage-A loads so the AllGather fires ~1/3 into the run.
  - all PE transposes use a bf16 identity as the moving operand (1.0
    cycles/row instead of 2.0 for f32, exact) on f32r-bitcast data.
  - conv = 36 shifted matmuls per (window, dc): stationary weights in bf16,
    moving activations f32r (full rate either way); gateBias is added by DVE
    from PSUM (no PE identity-matmul), ACT applies tanh into bf16 g-tiles.
  - the gate gather/transpose block sits after batch-3's convs in the PE
    stream, so the collective's fixed latency hides under conv work;
    elementwise for batches 0-3 runs right after the gates, batches 4-7
    inline. Outputs are stored per conv window to shrink the tail.
Outputs leave the chip transposed ([dc, 128, pix]); the host reassembles.
"""
import numpy as np

import bass_rust
import concourse.bass as bass
import concourse.mybir as mybir
import concourse.tile as tile
from concourse.bass_utils import run_bass_kernel_spmd

F32 = mybir.dt.float32
F32R = mybir.dt.float32r
BF16 = mybir.dt.bfloat16
I32 = mybir.dt.int32
AF = mybir.ActivationFunctionType
ALU = mybir.AluOpType

N_CORES = 8
B, H, W, CIN, D = 64, 28, 28, 256, 256
BL = B // N_CORES          # local batches per core
PIX = H * W                # 784
PG = 112                   # pixels per transpose group (4 rows)
NPG = PIX // PG            # 7
PAD = 30                   # padded row/col length
XTLEN = PAD * PAD          # 900
NW = 2                     # windows per batch
WROWS = H // NW            # 14
WN = WROWS * W             # 392
NCC = CIN // 128           # 2 channel chunks
NDC = D // 128             # 2 output-channel chunks

# tap order t = 3*kh + kw ; dh = kh-1, dw = kw-1
TAPS = [(kh, kw) for kh in range(3) for kw in range(3)]

# gapI tap expansion: (mask group, taps). Groups 1-4 are negative; the sign
# lives in the raw sums (negated at stage-A finalize).
GAP_TERMS = [(0, list(range(9))), (1, [6, 7, 8]), (2, [0, 1, 2]),
             (3, [2, 5, 8]), (4, [0, 3, 6]),
             (5, [8]), (6, [6]), (7, [2]), (8, [0])]

# ---------------------------------------------------------------------------
# walrus fixup: split semaphore waits that exceed the per-instruction budget
# (observed: Drain and Matmult accept only 1 semaphore wait each).
MAX_WAITS = 1


def _split_excess_sem_waits(nc):
    counter = [0]
    for fn in nc.m.functions:
        for bb in fn.blocks:
            insts = bb.instructions
            i = 0
            while i < len(insts):
                inst = insts[i]
                si = inst.sync_info
                if si is not None and si.on_wait and len(si.on_wait) > MAX_WAITS:
                    waits = list(si.on_wait)
                    excess = waits[:-MAX_WAITS]
                    keep = waits[-MAX_WAITS:]
                    new_insts = []
                    for j in range(0, len(excess), MAX_WAITS):
                        chunk = excess[j:j + MAX_WAITS]
                        noop = mybir.InstNoOp(
                            name=f"I-waitsplit-{counter[0]}", ins=[], outs=[])
                        counter[0] += 1
                        noop.engine = inst.engine
                        noop.sync_info = bass_rust.SyncInfo(
                            on_wait=chunk, on_update=[])
                        nc.register_instruction(noop)
                        new_insts.append(noop)
                    inst.sync_info = bass_rust.SyncInfo(
                        on_wait=keep, on_update=list(si.on_update))
                    insts[i:i] = new_insts
                    i += len(new_insts)
                i += 1
    return nc


# ---------------------------------------------------------------------------
def _r3(ap, w):
    """view flat free dim as (rows, w)"""
    return ap.rearrange("c (r w) -> c r w", w=w)


def build_nc():
    nc = bass.Bass("TRN2", target_bir_lowering=False, debug=False,
                   num_devices=N_CORES)

    dram = {}
    dram["inputs"] = nc.dram_tensor("inputs", [BL, PIX, CIN], BF16,
                                    kind="ExternalInput").ap()
    dram["state_prev"] = nc.dram_tensor("state_prev", [BL, PIX, D], BF16,
                                        kind="ExternalInput").ap()
    dram["hidden_prev"] = nc.dram_tensor("hidden_prev", [BL, PIX, D], BF16,
                                         kind="ExternalInput").ap()
    dram["wpk"] = nc.dram_tensor("wconv_bf16", [128, 2 * 9 * NCC * 256],
                                 BF16, kind="ExternalInput").ap()
    dram["gb"] = nc.dram_tensor("gateBias", [PIX, D], F32,
                                kind="ExternalInput").ap()
    dram["vec"] = {}
    for nm in ("wxi", "whi", "inputBias", "wxf", "whf", "forgetBias",
               "wxo", "who", "outputBias"):
        dram["vec"][nm] = nc.dram_tensor(nm, [D, 1], F32,
                                         kind="ExternalInput").ap()
    dram["idx"] = nc.dram_tensor("gate_idx", [4, 224, 1], I32,
                                 kind="ExternalInput").ap()
    dram["masks"] = nc.dram_tensor("gap_masks", [NPG, PG, 9], BF16,
                                   kind="ExternalInput").ap()
    dram["hidden"] = nc.dram_tensor("hidden", [BL, NDC, 128, PIX], F32,
                                    kind="ExternalOutput").ap()
    dram["state"] = nc.dram_tensor("state", [BL, NDC, 128, PIX], F32,
                                   kind="ExternalOutput").ap()
    dram["cc_in"] = nc.dram_tensor("cc_in", [32, 128], F32, kind="Internal").ap()
    dram["cc_out"] = nc.dram_tensor("cc_out", [N_CORES * 32, 128], F32,
                                    kind="Internal", addr_space="Shared").ap()

    ctx_mgr = nc.allow_low_precision("bf16 weights + float32r operands for PE")
    ctx_mgr.__enter__()
    with tile.TileContext(nc) as tc:
        _build_body(nc, tc, dram)
    ctx_mgr.__exit__(None, None, None)
    return nc


def _build_body(nc, tc, dram):
    from collections import deque
    from contextlib import ExitStack
    ctx = ExitStack()
    pool = lambda **kw: ctx.enter_context(tc.tile_pool(**kw))

    const = pool(name="const", bufs=1)
    wts = pool(name="wts", bufs=1)
    wst = pool(name="wst", bufs=1)         # f32 weight / gateBias staging
    natb = pool(name="natb", bufs=3)       # stage-B natural loads (in/hid)
    nata = pool(name="nata", bufs=2)       # stage-A-only loads (j=3..7)
    natsp = pool(name="natsp", bufs=4)     # state_prev natural loads
    xt_in = pool(name="xt_in", bufs=2)
    xt_hid = pool(name="xt_hid", bufs=2)
    xt_sp = pool(name="xt_sp", bufs=5)
    ew = pool(name="ew", bufs=2)
    gtp = pool(name="gtp", bufs=20)        # bf16 g tiles (deferred ew b0-3)
    outb = pool(name="outb", bufs=3)
    gsm = pool(name="gsm", bufs=1)
    gtmp = pool(name="gtmp", bufs=2)
    ps_conv = pool(name="ps_conv", bufs=3, space="PSUM")
    ps_tr = pool(name="ps_tr", bufs=3, space="PSUM")
    ps_gap = pool(name="ps_gap", bufs=1, space="PSUM")

    # ---- constants (deadline-ordered: b0 data + identity first) ----
    idx_sb = []
    vecs = {}

    def load_idx_and_vecs():
        for g4 in range(4):
            halves = []
            for hf in range(2):
                t = const.tile([PG, 1], I32, tag=f"idx{g4}_{hf}")
                nc.sync.dma_start(t[:],
                                  dram["idx"][g4, hf * PG:(hf + 1) * PG, :])
                halves.append(t)
            idx_sb.append(halves)
        for nm in dram["vec"]:
            t = const.tile([128, NDC], F32, tag=f"vec_{nm}")
            for c in range(NDC):
                nc.scalar.dma_start(t[:, c:c + 1],
                                    dram["vec"][nm][c * 128:(c + 1) * 128, :])
            if nm.startswith("wx") or nm.startswith("wh"):
                nc.vector.tensor_scalar_mul(t[:], t[:], 1.0 / PIX)
            vecs[nm] = t

    # ---- conv weights: f32 staged -> bf16 resident ----
    wconv = wts.tile([128, 2 * 9 * NCC * 256], BF16, tag="wconv")

    def wblk(conv, t, cc):
        off = ((conv * 9 + t) * NCC + cc) * 256
        return wconv[:, off:off + 256]

    def load_weights():
        half = 9 * NCC * 256
        nc.scalar.dma_start(wconv[:, 0:half], dram["wpk"][:, 0:half])
        nc.scalar.dma_start(wconv[:, half:2 * half],
                            dram["wpk"][:, half:2 * half])

    # gateBias: one big load, transposed to [128 d, PIX] bf16 per dc
    gbias = [const.tile([128, PIX], BF16, tag=f"gbias{dc}", name=f"gbias{dc}")
             for dc in range(NDC)]

    # stage-A accumulators
    raw = [gsm.tile([128, 9 * BL], BF16, tag=f"raw{cc}", name=f"raw{cc}")
           for cc in range(NCC)]
    gapH = [gsm.tile([128, BL], F32, tag=f"gapH{cc}", name=f"gapHs{cc}")
            for cc in range(NCC)]

    # ---- natural loads -----------------------------------------------------
    natI, natH, natS = {}, {}, {}     # stage-B tiles
    aI, aH = {}, {}                   # stage-A-only tiles (j=3..7)

    def load_nat(j, which):
        """one 3D DMA: [112, 7, 256] natural-layout tile"""
        if which == "in":
            src, p, tag = dram["inputs"], natb, "natb"
        elif which == "hid":
            src, p, tag = dram["hidden_prev"], natb, "natb"
        elif which == "ain":
            src, p, tag = dram["inputs"], nata, "nata"
        elif which == "ahid":
            src, p, tag = dram["hidden_prev"], nata, "nata"
        else:
            src, p, tag = dram["state_prev"], natsp, "natsp"
        t = p.tile([PG, NPG, 256], BF16, tag=tag, name=f"nat_{which}_{j}")
        q = nc.scalar if which == "hid" else nc.sync
        q.dma_start(t[:], src[j].rearrange("(g p) d -> p g d", p=PG))
        return t

    # ---- stage A: masked pixel sums off a natural tile ---------------------
    def emit_stage_a(j, nat, is_input):
        p9 = ps_gap.tile([9, 256], F32, tag="rawT")
        for pg in range(NPG):
            nc.tensor.matmul(p9[:], masks[pg][:],
                             nat[:, pg, :],
                             start=(pg == 0), stop=(pg == NPG - 1))
        rt = gtmp.tile([9, 256], F32, tag="rawT_sb", bufs=1)
        nc.vector.tensor_copy(rt[:], p9[:])
        for cc in range(NCC):
            cs = slice(cc * 128, (cc + 1) * 128)
            ptw = ps_tr.tile([128, PG], F32, tag="ptr")
            pt = ptw[:, 0:9]
            nc.tensor.transpose(pt, rt[:, cs], ident[0:9, 0:9])
            if is_input:
                r3 = _r3(raw[cc][:], BL)
                nc.vector.tensor_copy(
                    r3[:, :, j:j + 1],
                    pt.rearrange("c (n o) -> c n o", o=1))
                # fold the negative sign of mask groups 1-4 into raw
                nc.vector.tensor_scalar_mul(r3[:, 1:5, j:j + 1],
                                            r3[:, 1:5, j:j + 1], -1.0)
            else:
                nc.vector.tensor_copy(gapH[cc][:, j:j + 1], pt[:, 0:1])

    # ---- stage B: transposes into padded conv layout -----------------------
    xin, xhid, xsp = {}, {}, {}

    def alloc_xt(j, which):
        if which == "in":
            p, dst, tag = xt_in, xin, "xin"
        elif which == "hid":
            p, dst, tag = xt_hid, xhid, "xhid"
        else:
            p, dst, tag = xt_sp, xsp, "xsp"
        tiles = []
        for cc in range(NCC):
            xlen = PIX if which == "sp" else XTLEN
            xt = p.tile([128, xlen], BF16, tag=f"{tag}{cc}",
                        name=f"{tag}{cc}_{j}")
            if which != "sp":
                x3 = _r3(xt[:], PAD)
                nc.gpsimd.memset(x3[:, 0:1, :], 0.0)
                nc.gpsimd.memset(x3[:, PAD - 1:PAD, :], 0.0)
                nc.gpsimd.memset(x3[:, 1:PAD - 1, 0:1], 0.0)
                nc.gpsimd.memset(x3[:, 1:PAD - 1, PAD - 1:PAD], 0.0)
            tiles.append(xt)
        dst[j] = tiles
        return tiles

    def emit_transpose_group(nat, tiles, g7, padded):
        for cc in range(NCC):
            if padded:
                pt = ps_tr.tile([128, PG], BF16, tag="ptr")
                nc.tensor.transpose(pt[:],
                                    nat[:, g7, cc * 128:(cc + 1) * 128],
                                    identb[0:PG, 0:PG])
                dst = _r3(tiles[cc][:], PAD)[:, 1 + 4 * g7:1 + 4 * g7 + 4,
                                             1:29]
                nc.vector.tensor_copy(dst,
                                      pt[:].rearrange("c (r w) -> c r w", w=W))
            else:
                pt = ps_tr.tile([128, PG], BF16, tag="ptr")
                nc.tensor.transpose(pt[:],
                                    nat[:, g7, cc * 128:(cc + 1) * 128],
                                    identb[0:PG, 0:PG])
                dst = _r3(tiles[cc][:, g7 * PG:(g7 + 1) * PG], W)
                nc.vector.tensor_copy(dst,
                                      pt[:].rearrange("c (r w) -> c r w", w=W))

    # ---- conv window: 36 matmuls + DVE bias add + ACT tanh -----------------
    def emit_conv_window(j, wi, dc, r0=None, nr=WROWS):
        if r0 is None:
            r0 = wi * WROWS
        h0 = 1 + r0
        base = r0 * W
        wn = nr * W
        p = ps_conv.tile([128, WN], F32, tag="pconv", name="pconv")
        p3 = _r3(p[:, 0:wn], W)
        first = True
        for conv, xbuf in ((0, xin[j]), (1, xhid[j])):
            for t, (kh, kw) in enumerate(TAPS):
                dh, dwid = kh - 1, kw - 1
                for cc in range(NCC):
                    rhs = _r3(xbuf[cc][:], PAD)[
                        :, h0 + dh:h0 + dh + nr, 1 + dwid:1 + dwid + W]
                    last = (conv == 1 and t == 8 and cc == NCC - 1)
                    nc.tensor.matmul(
                        p3, wblk(conv, t, cc)[:, dc * 128:(dc + 1) * 128],
                        rhs, start=first, stop=last)
                    first = False
        pre = ew.tile([128, WN], F32, tag="pre", bufs=3, name="pre")
        nc.vector.tensor_tensor(out=pre[:, 0:wn], in0=p[:, 0:wn],
                                in1=gbias[dc][:, base:base + wn], op=ALU.add)
        gt = gtp.tile([128, WN], BF16, tag="gt", name=f"gt{j}_{wi}_{dc}")
        nc.scalar.activation(gt[:, 0:wn], pre[:, 0:wn], AF.Tanh)
        return gt

    # ---- elementwise + per-window store ------------------------------------
    gates = {}

    def emit_elementwise(j, wi, dc, gt, r0=None, nr=WROWS):
        if r0 is None:
            r0 = wi * WROWS
        base = r0 * W
        wn = nr * W
        t0 = j * H + r0

        def gw(gate):
            return gates[gate][dc][:, t0:t0 + nr].to_broadcast(
                [128, nr, W])

        sp3 = _r3(xsp[j][dc][:, base:base + wn], W)
        g3 = _r3(gt[:, 0:wn], W)
        s1 = ew.tile([128, WN], F32, tag="s1", name="s1")
        nc.gpsimd.tensor_tensor(out=_r3(s1[:, 0:wn], W), in0=sp3, in1=gw("f"),
                                op=ALU.mult)
        s2 = ew.tile([128, WN], F32, tag="s2", name="s2")
        nc.gpsimd.tensor_tensor(out=_r3(s2[:, 0:wn], W), in0=g3, in1=gw("i"),
                                op=ALU.mult)
        st = outb.tile([128, WN], F32, tag="st", name=f"st{j}_{wi}_{dc}")
        nc.gpsimd.tensor_tensor(out=_r3(st[:, 0:wn], W),
                                in0=_r3(s1[:, 0:wn], W),
                                in1=_r3(s2[:, 0:wn], W), op=ALU.add)
        th = ew.tile([128, WN], F32, tag="th", name="th")
        nc.scalar.activation(th[:, 0:wn], st[:, 0:wn], AF.Tanh)
        hd = outb.tile([128, WN], F32, tag="hd", name=f"hd{j}_{wi}_{dc}")
        nc.gpsimd.tensor_tensor(out=_r3(hd[:, 0:wn], W),
                                in0=_r3(th[:, 0:wn], W),
                                in1=gw("o"), op=ALU.mult)
        nc.sync.dma_start(dram["state"][j, dc][:, base:base + wn],
                          st[:, 0:wn])
        nc.scalar.dma_start(dram["hidden"][j, dc][:, base:base + wn],
                            hd[:, 0:wn])

    # ---- PE-side work queue (drained between conv windows) -----------------
    peq = deque()

    def drain(n):
        for _ in range(min(n, len(peq))):
            peq.popleft()()

    def queue_batch_transposes(j):
        ti = alloc_xt(j, "in")
        th_ = alloc_xt(j, "hid")
        groups = []
        for g7 in range(NPG):
            groups.append(lambda g7=g7, j=j, ti=ti: emit_transpose_group(
                natI[j], ti, g7, True))
            groups.append(lambda g7=g7, j=j, th_=th_: emit_transpose_group(
                natH[j], th_, g7, True))
        if j == 0:
            groups = groups[0::2] + groups[1::2]
        peq.extend(groups)

    def queue_sp_transposes(j):
        ts = alloc_xt(j, "sp")
        for g7 in range(NPG):
            peq.append(lambda g7=g7, j=j, ts=ts: emit_transpose_group(
                natS[j], ts, g7, False))

    def queue_stage_a(j):
        if j <= 2:
            peq.append(lambda j=j: emit_stage_a(j, natI[j], True))
            peq.append(lambda j=j: emit_stage_a(j, natH[j], False))
        else:
            peq.append(lambda j=j: emit_stage_a(j, aI[j], True))
            peq.append(lambda j=j: emit_stage_a(j, aH[j], False))

    # ---- gap combine + collective ------------------------------------------
    def emit_gap_combine_and_allgather():
        gap_ps = ps_gap.tile([8, 256], F32, tag="gapI")
        terms = [(g, t, cc) for g, taps in GAP_TERMS for t in taps
                 for cc in range(NCC)]
        for i, (g, t, cc) in enumerate(terms):
            nc.tensor.matmul(gap_ps[:], raw[cc][:, g * BL:(g + 1) * BL],
                             wblk(0, t, cc),
                             start=(i == 0), stop=(i == len(terms) - 1))
        gapI_sb = gsm.tile([8, 256], F32, tag="gapI_sb")
        nc.vector.tensor_copy(gapI_sb[:], gap_ps[:])
        nc.sync.dma_start(dram["cc_in"][0:8, :], gapI_sb[:, 0:128])
        nc.sync.dma_start(dram["cc_in"][8:16, :], gapI_sb[:, 128:256])
        for cc in range(NCC):
            pt = ps_gap.tile([8, 128], F32, tag="gapI")
            nc.tensor.transpose(pt[:], gapH[cc][:], ident[:])
            hs = gsm.tile([8, 128], F32, tag=f"gapH_sb{cc}",
                          name=f"gapHsb{cc}")
            nc.vector.tensor_copy(hs[:], pt[:])
            nc.sync.dma_start(dram["cc_in"][16 + 8 * cc:24 + 8 * cc, :], hs[:])
        nc.gpsimd.collective_compute(
            "AllGather", ALU.bypass, replica_groups=[list(range(N_CORES))],
            ins=[dram["cc_in"][:]], outs=[dram["cc_out"][:]])

    def emit_gather_and_gates():
        sel = [gsm.tile([128, 224], BF16, tag=f"sel{g4}", name=f"sel{g4}")
               for g4 in range(4)]
        for g4 in range(4):
            for hf in range(2):
                gtile = gtmp.tile([PG, 128], F32, tag="gath", name="gath")
                nc.gpsimd.indirect_dma_start(
                    out=gtile[:], out_offset=None, in_=dram["cc_out"][:],
                    in_offset=bass.IndirectOffsetOnAxis(
                        ap=idx_sb[g4][hf][:, :1], axis=0))
                pt = ps_tr.tile([128, PG], F32, tag="ptr", name="pt_gath")
                nc.tensor.transpose(pt[:], gtile[:],
                                    ident[0:PG, 0:PG])
                nc.vector.tensor_copy(sel[g4][:, hf * PG:(hf + 1) * PG],
                                      pt[:])
        for gate, wx, wh, bi in (("i", "wxi", "whi", "inputBias"),
                                 ("f", "wxf", "whf", "forgetBias"),
                                 ("o", "wxo", "who", "outputBias")):
            per_dc = []
            for dc in range(NDC):
                t1 = gtmp.tile([128, 224], F32, tag="gm1", bufs=1, name="gm1")
                nc.vector.tensor_scalar_mul(t1[:], sel[dc][:],
                                            vecs[wx][:, dc:dc + 1])
                t2 = gtmp.tile([128, 224], F32, tag="gm2", bufs=1, name="gm2")
                nc.vector.tensor_scalar_mul(t2[:], sel[2 + dc][:],
                                            vecs[wh][:, dc:dc + 1])
                nc.vector.tensor_tensor(out=t1[:], in0=t1[:], in1=t2[:],
                                        op=ALU.add)
                gtb = gsm.tile([128, 224], F32, tag=f"gate_{gate}{dc}",
                               name=f"gate_{gate}{dc}")
                nc.scalar.activation(gtb[:], t1[:], AF.Sigmoid,
                                     bias=vecs[bi][:, dc:dc + 1])
                per_dc.append(gtb)
            gates[gate] = per_dc

    # ======================= emission schedule ==============================
    # b0 stage-B loads + identity first on the sync queue; weights first on
    # the act queue. Stage-A-only loads are paced in the slot loop (their DMA
    # dispatch can block the sync queue on buffer-free waits, so nothing
    # deadline-critical may follow them in the same slot).
    natI[0] = load_nat(0, "in")
    natH[0] = load_nat(0, "hid")
    # identity built on-chip: iota(p - i) == 0 selects the diagonal
    ident = const.tile([128, 128], F32, tag="ident")
    nc.gpsimd.memset(ident[:], 1.0)
    nc.gpsimd.affine_select(ident[:], ident[:], [[-1, 128]],
                            ALU.is_equal, 0.0, base=0, channel_multiplier=1)
    identb = const.tile([128, 128], BF16, tag="identb")
    nc.vector.tensor_copy(identb[:], ident[:])
    load_weights()
    masks = []
    for pg in range(NPG):
        m = const.tile([PG, 9], BF16, tag=f"mask{pg}", name=f"mask{pg}")
        nc.sync.dma_start(m[:], dram["masks"][pg])
        masks.append(m)
    natI[1] = load_nat(1, "in")
    natH[1] = load_nat(1, "hid")
    gnat = wst.tile([PG, NPG, 256], F32, tag="gnat", bufs=1)
    nc.sync.dma_start(gnat[:],
                      dram["gb"].rearrange("(g p) d -> p g d", p=PG))
    load_idx_and_vecs()

    def emit_gbias_group(g7):
        for dc in range(NDC):
            pt = ps_tr.tile([128, PG], F32, tag="ptr")
            nc.tensor.transpose(
                pt[:], gnat[:, g7, dc * 128:(dc + 1) * 128],
                ident[0:PG, 0:PG])
            nc.vector.tensor_copy(gbias[dc][:, g7 * PG:(g7 + 1) * PG],
                                  pt[:])

    # prologue PE work, drained now: batch-0 transposes + stage A for b0 +
    # gbias transposes (the first conv window's bias-add reads gbias, so it
    # must be written before slot 0 in DVE program order).
    queue_batch_transposes(0)
    queue_stage_a(0)
    for g7 in range(NPG):
        peq.append(lambda g7=g7: emit_gbias_group(g7))
    drain(len(peq))
    queue_batch_transposes(1)
    queue_stage_a(1)

    gts = {}
    for k in range(BL):
        # just-in-time loads (stage-B two batches ahead; stage-A paced)
        if k + 2 < BL:
            natI[k + 2] = load_nat(k + 2, "in")
            natH[k + 2] = load_nat(k + 2, "hid")
        if k == 0:
            for j in (3, 4):
                aI[j] = load_nat(j, "ain")
                aH[j] = load_nat(j, "ahid")
        elif k == 2:
            for j in (5, 6, 7):
                aI[j] = load_nat(j, "ain")
                aH[j] = load_nat(j, "ahid")
            for j in range(0, 4):
                natS[j] = load_nat(j, "sp")
        elif k >= 3 and k + 1 < BL:
            natS[k + 1] = load_nat(k + 1, "sp")

        # queue PE side work for this slot (b1 already queued in prologue)
        if 1 <= k and k + 1 < BL:
            queue_batch_transposes(k + 1)
        if k == 0:
            queue_stage_a(2)
        elif k == 1:
            queue_stage_a(3)
            queue_stage_a(4)
        elif k == 2:
            queue_stage_a(5)
            queue_stage_a(6)
            queue_stage_a(7)
        if k >= 5 and k + 1 < BL:
            queue_sp_transposes(k + 1)

        nwin = NW * NDC
        for wi in range(NW):
            for dc in range(NDC):
                halves = (k == BL - 1 and wi == NW - 1 and dc == NDC - 1)
                if halves:
                    hw_ = WROWS // 2
                    for hh in range(2):
                        gt = emit_conv_window(k, wi, dc,
                                              r0=wi * WROWS + hh * hw_,
                                              nr=hw_)
                        drain(len(peq))
                        emit_elementwise(k, wi, dc, gt,
                                         r0=wi * WROWS + hh * hw_, nr=hw_)
                    continue
                gt = emit_conv_window(k, wi, dc)
                rem = len(peq)
                left = nwin - (wi * NDC + dc)
                drain(max(5, (rem + left - 1) // left))
                # ew is emitted after the drain so its sp-layout transposes
                # (front of the queue at k>=5) precede it in DVE order
                if k <= 4:
                    gts[(k, wi, dc)] = gt
                else:
                    emit_elementwise(k, wi, dc, gt)

        if k == 2:
            drain(len(peq))           # all stage A must be emitted
            emit_gap_combine_and_allgather()
        if k == 4:
            drain(len(peq))
            emit_gather_and_gates()
            for j in range(0, 4):
                queue_sp_transposes(j)
            drain(len(peq))
            for j in range(0, 2):
                for wi in range(NW):
                    for dc in range(NDC):
                        emit_elementwise(j, wi, dc, gts.pop((j, wi, dc)))
            queue_sp_transposes(4)
            drain(len(peq))
            for j in range(2, 5):
                for wi in range(NW):
                    for dc in range(NDC):
                        emit_elementwise(j, wi, dc, gts.pop((j, wi, dc)))
            queue_sp_transposes(5)
    drain(len(peq))

    ctx.close()


# ---------------------------------------------------------------------------
_NC_CACHE = None


def _get_nc():
    global _NC_CACHE
    if _NC_CACHE is None:
        nc = build_nc()
        _split_excess_sem_waits(nc)
        _NC_CACHE = nc
    return _NC_CACHE


def _gate_idx(core):
    idx = np.empty((4, 224, 1), np.int32)
    for j in range(BL):
        for hh in range(H):
            t = j * H + hh
            sel_b = (H * (BL * core + j) + hh) % B
            cp, bp = sel_b // BL, sel_b % BL
            for g in range(4):
                idx[g, t, 0] = cp * 32 + g * 8 + bp
    return idx


def _gap_masks():
    m = np.zeros((PIX, 9), np.float32)
    hw = np.arange(PIX)
    r, c = hw // W, hw % W
    m[:, 0] = 1.0
    m[r == 0, 1] = 1.0
    m[r == H - 1, 2] = 1.0
    m[c == 0, 3] = 1.0
    m[c == W - 1, 4] = 1.0
    m[(r == 0) & (c == 0), 5] = 1.0
    m[(r == 0) & (c == W - 1), 6] = 1.0
    m[(r == H - 1) & (c == 0), 7] = 1.0
    m[(r == H - 1) & (c == W - 1), 8] = 1.0
    return m.reshape(NPG, PG, 9)


def _make_in_maps(inputs):
    f32 = np.float32
    import ml_dtypes
    wpk = np.empty((128, 2 * 9 * NCC * 256), dtype=ml_dtypes.bfloat16)
    for conv, w in ((0, np.asarray(inputs["wconvInput"], dtype=f32)),
                    (1, np.asarray(inputs["wconvHidden"], dtype=f32))):
        for t, (kh, kw) in enumerate(TAPS):
            for cc in range(NCC):
                off = ((conv * 9 + t) * NCC + cc) * 256
                wpk[:, off:off + 256] = w[kh, kw,
                                          cc * 128:(cc + 1) * 128, :]
    shared = {
        "wconv_bf16": wpk,
        "gateBias": np.ascontiguousarray(inputs["gateBias"],
                                         dtype=f32).reshape(PIX, D),
        "gap_masks": _gap_masks().astype(ml_dtypes.bfloat16),
    }
    for nm in ("wxi", "whi", "inputBias", "wxf", "whf", "forgetBias",
               "wxo", "who", "outputBias"):
        shared[nm] = np.ascontiguousarray(inputs[nm], dtype=f32).reshape(D, 1)

    bf16 = ml_dtypes.bfloat16
    xin = np.ascontiguousarray(inputs["inputs"], dtype=f32).reshape(
        B, PIX, CIN).astype(bf16)
    xsp = np.ascontiguousarray(inputs["state_prev"], dtype=f32).reshape(
        B, PIX, D).astype(bf16)
    xhp = np.ascontiguousarray(inputs["hidden_prev"], dtype=f32).reshape(
        B, PIX, D).astype(bf16)

    in_maps = []
    for k in range(N_CORES):
        sl = slice(k * BL, (k + 1) * BL)
        m = dict(shared)
        m["inputs"] = xin[sl]
        m["state_prev"] = xsp[sl]
        m["hidden_prev"] = xhp[sl]
        m["gate_idx"] = _gate_idx(k)
        in_maps.append(m)
    return in_maps


def kernel(**inputs):
    nc = _get_nc()
    in_maps = _make_in_maps(inputs)
    res = run_bass_kernel_spmd(nc, in_maps, core_ids=list(range(N_CORES)))

    def unshard(name):
        # per-core outputs are [BL, NDC, 128, PIX] (channel-major); restore NHWC
        full = np.concatenate([res.results[k][name] for k in range(N_CORES)],
                              axis=0)
        return np.ascontiguousarray(full.transpose(0, 3, 1, 2)).reshape(
            B, H, W, D)

    return unshard("hidden"), unshard("state")


# revision 28
# speedup vs baseline: 1.0684x; 1.0148x over previous
"""Trainium2 Bass kernel for nn_DeformableConvLSTMCell_33895881900284.

Full (unsharded) inputs in, full outputs out. Internally: data-parallel over
batch across 8 NeuronCores (8 batches per core), conv weights / gate params
replicated.

Math per the reference:
  outI  = conv3x3_same(inputs, wconvInput)
  g     = tanh(outI + conv3x3_same(hidden_prev, wconvHidden) + gateBias)
  gapI  = mean_hw(outI);  gapH = mean_hw(hidden_prev)          # [B, D]
  i/f/o = sigmoid(wx*gapI + wh*gapH + bias)                    # [B, D]
  tiled gate: value used at (b, h, w, c) is gate[(28*b + h) % 64, c]
  state  = f*state_prev + i*g;  hidden = o*tanh(state)

The (28*b+h)%64 scrambling makes gates cross-batch: each core computes its
local GAP columns, all cores AllGather them, and a per-core index-array input
drives an indirect-DMA gather of exactly the gate rows this core's outputs
need (the SPMD program stays identical across cores; only input data differs).

gapI never touches the conv output: by linearity, 784*gapI is a combination
of 9 masked pixel sums of the raw input (full sum, edge rows/cols, corners)
matmul'd with summed conv-weight taps ("stage A"). Groups 1-4 carry negative
coefficients; the sign is folded into the raw sums so the final combine is a
pure PSUM accumulation over individual weight taps (no combined-A tile).

Pipeline design (PE is the bottleneck engine, keep it saturated):
  - one big 3D DMA per (batch, tensor): [112, 7, 256]; stage-A masked-sum
    matmuls ride the same natural tiles for batches 0-2; batches 3-7 get
    dedicated paced stage-A loads so the AllGather fires ~1/3 into the run.
  - all PE transposes use a bf16 identity as the moving operand (1.0
    cycles/row instead of 2.0 for f32, exact) on f32r-bitcast data.
  - conv = 36 shifted matmuls per (window, dc): stationary weights in bf16,
    moving activations f32r (full rate either way); gateBias is added by DVE
    from PSUM (no PE identity-matmul), ACT applies tanh into bf16 g-tiles.
  - the gate gather/transpose block sits after batch-3's convs in the PE
    stream, so the collective's fixed latency hides under conv work;
    elementwise for batches 0-3 runs right after the gates, batches 4-7
    inline. Outputs are stored per conv window to shrink the tail.
Outputs leave the chip transposed ([dc, 128, pix]); the host reassembles.
"""
import numpy as np

import bass_rust
import concourse.bass as bass
import concourse.mybir as mybir
import concourse.tile as tile
from concourse.bass_utils import run_bass_kernel_spmd

F32 = mybir.dt.float32
F32R = mybir.dt.float32r
BF16 = mybir.dt.bfloat16
I32 = mybir.dt.int32
AF = mybir.ActivationFunctionType
ALU = mybir.AluOpType

N_CORES = 8
B, H, W, CIN, D = 64, 28, 28, 256, 256
BL = B // N_CORES          # local batches per core
PIX = H * W                # 784
PG = 112                   # pixels per transpose group (4 rows)
NPG = PIX // PG            # 7
PAD = 30                   # padded row/col length
XTLEN = PAD * PAD          # 900
NW = 2                     # windows per batch
WROWS = H // NW            # 14
WN = WROWS * W             # 392
NCC = CIN // 128           # 2 channel chunks
NDC = D // 128             # 2 output-channel chunks

# tap order t = 3*kh + kw ; dh = kh-1, dw = kw-1
TAPS = [(kh, kw) for kh in range(3) for kw in range(3)]

# gapI tap expansion: (mask group, taps). Groups 1-4 are negative; the sign
# lives in the raw sums (negated at stage-A finalize).
GAP_TERMS = [(0, list(range(9))), (1, [6, 7, 8]), (2, [0, 1, 2]),
             (3, [2, 5, 8]), (4, [0, 3, 6]),
             (5, [8]), (6, [6]), (7, [2]), (8, [0])]

# ---------------------------------------------------------------------------
# walrus fixup: split semaphore waits that exceed the per-instruction budget
# (observed: Drain and Matmult accept only 1 semaphore wait each).
MAX_WAITS = 1


def _split_excess_sem_waits(nc):
    counter = [0]
    for fn in nc.m.functions:
        for bb in fn.blocks:
            insts = bb.instructions
            i = 0
            while i < len(insts):
                inst = insts[i]
                si = inst.sync_info
                if si is not None and si.on_wait and len(si.on_wait) > MAX_WAITS:
                    waits = list(si.on_wait)
                    excess = waits[:-MAX_WAITS]
                    keep = waits[-MAX_WAITS:]
                    new_insts = []
                    for j in range(0, len(excess), MAX_WAITS):
                        chunk = excess[j:j + MAX_WAITS]
                        noop = mybir.InstNoOp(
                            name=f"I-waitsplit-{counter[0]}", ins=[], outs=[])
                        counter[0] += 1
                        noop.engine = inst.engine
                        noop.sync_info = bass_rust.SyncInfo(
                            on_wait=chunk, on_update=[])
                        nc.register_instruction(noop)
                        new_insts.append(noop)
                    inst.sync_info = bass_rust.SyncInfo(
                        on_wait=keep, on_update=list(si.on_update))
                    insts[i:i] = new_insts
                    i += len(new_insts)
                i += 1
    return nc


# ---------------------------------------------------------------------------
def _r3(ap, w):
    """view flat free dim as (rows, w)"""
    return ap.rearrange("c (r w) -> c r w", w=w)


def build_nc():
    nc = bass.Bass("TRN2", target_bir_lowering=False, debug=False,
                   num_devices=N_CORES)

    dram = {}
    dram["inputs"] = nc.dram_tensor("inputs", [BL, PIX, CIN], BF16,
                                    kind="ExternalInput").ap()
    dram["state_prev"] = nc.dram_tensor("state_prev", [BL, PIX, D], BF16,
                                        kind="ExternalInput").ap()
    dram["hidden_prev"] = nc.dram_tensor("hidden_prev", [BL, PIX, D], BF16,
                                         kind="ExternalInput").ap()
    dram["wpk"] = nc.dram_tensor("wconv_bf16", [128, 2 * 9 * NCC * 256],
                                 BF16, kind="ExternalInput").ap()
    dram["gb"] = nc.dram_tensor("gateBias", [PIX, D], F32,
                                kind="ExternalInput").ap()
    dram["vec"] = {}
    for nm in ("wxi", "whi", "inputBias", "wxf", "whf", "forgetBias",
               "wxo", "who", "outputBias"):
        dram["vec"][nm] = nc.dram_tensor(nm, [D, 1], F32,
                                         kind="ExternalInput").ap()
    dram["idx"] = nc.dram_tensor("gate_idx", [4, 224, 1], I32,
                                 kind="ExternalInput").ap()
    dram["masks"] = nc.dram_tensor("gap_masks", [NPG, PG, 9], BF16,
                                   kind="ExternalInput").ap()
    dram["hidden"] = nc.dram_tensor("hidden", [BL, NDC, 128, PIX], F32,
                                    kind="ExternalOutput").ap()
    dram["state"] = nc.dram_tensor("state", [BL, NDC, 128, PIX], F32,
                                   kind="ExternalOutput").ap()
    dram["cc_in"] = nc.dram_tensor("cc_in", [32, 128], F32, kind="Internal").ap()
    dram["cc_out"] = nc.dram_tensor("cc_out", [N_CORES * 32, 128], F32,
                                    kind="Internal", addr_space="Shared").ap()

    ctx_mgr = nc.allow_low_precision("bf16 weights + float32r operands for PE")
    ctx_mgr.__enter__()
    with tile.TileContext(nc) as tc:
        _build_body(nc, tc, dram)
    ctx_mgr.__exit__(None, None, None)
    return nc


def _build_body(nc, tc, dram):
    from collections import deque
    from contextlib import ExitStack
    ctx = ExitStack()
    pool = lambda **kw: ctx.enter_context(tc.tile_pool(**kw))

    const = pool(name="const", bufs=1)
    wts = pool(name="wts", bufs=1)
    wst = pool(name="wst", bufs=1)         # f32 weight / gateBias staging
    natb = pool(name="natb", bufs=3)       # stage-B natural loads (in/hid)
    nata = pool(name="nata", bufs=2)       # stage-A-only loads (j=3..7)
    natsp = pool(name="natsp", bufs=4)     # state_prev natural loads
    xt_in = pool(name="xt_in", bufs=2)
    xt_hid = pool(name="xt_hid", bufs=2)
    xt_sp = pool(name="xt_sp", bufs=5)
    ew = pool(name="ew", bufs=2)
    gtp = pool(name="gtp", bufs=20)        # bf16 g tiles (deferred ew b0-3)
    outb = pool(name="outb", bufs=3)
    gsm = pool(name="gsm", bufs=1)
    gtmp = pool(name="gtmp", bufs=2)
    ps_conv = pool(name="ps_conv", bufs=3, space="PSUM")
    ps_tr = pool(name="ps_tr", bufs=3, space="PSUM")
    ps_gap = pool(name="ps_gap", bufs=1, space="PSUM")

    # ---- constants (deadline-ordered: b0 data + identity first) ----
    idx_sb = []
    vecs = {}

    def load_idx_and_vecs():
        for g4 in range(4):
            halves = []
            for hf in range(2):
                t = const.tile([PG, 1], I32, tag=f"idx{g4}_{hf}")
                nc.sync.dma_start(t[:],
                                  dram["idx"][g4, hf * PG:(hf + 1) * PG, :])
                halves.append(t)
            idx_sb.append(halves)
        for nm in dram["vec"]:
            t = const.tile([128, NDC], F32, tag=f"vec_{nm}")
            for c in range(NDC):
                nc.scalar.dma_start(t[:, c:c + 1],
                                    dram["vec"][nm][c * 128:(c + 1) * 128, :])
            if nm.startswith("wx") or nm.startswith("wh"):
                nc.vector.tensor_scalar_mul(t[:], t[:], 1.0 / PIX)
            vecs[nm] = t

    # ---- conv weights: f32 staged -> bf16 resident ----
    wconv = wts.tile([128, 2 * 9 * NCC * 256], BF16, tag="wconv")

    def wblk(conv, t, cc):
        off = ((conv * 9 + t) * NCC + cc) * 256
        return wconv[:, off:off + 256]

    def load_weights():
        half = 9 * NCC * 256
        nc.scalar.dma_start(wconv[:, 0:half], dram["wpk"][:, 0:half])
        nc.scalar.dma_start(wconv[:, half:2 * half],
                            dram["wpk"][:, half:2 * half])

    # gateBias: one big load, transposed to [128 d, PIX] bf16 per dc
    gbias = [const.tile([128, PIX], BF16, tag=f"gbias{dc}", name=f"gbias{dc}")
             for dc in range(NDC)]

    # stage-A accumulators
    raw = [gsm.tile([128, 9 * BL], BF16, tag=f"raw{cc}", name=f"raw{cc}")
           for cc in range(NCC)]
    acmb = wts.tile([128, NCC * 5 * 256], BF16, tag="acmb")

    def acblk(cc, g):
        off = (cc * 5 + g) * 256
        return acmb[:, off:off + 256]

    def emit_a_combine():
        for cc in range(NCC):
            nc.vector.tensor_copy(acblk(cc, 0), wblk(0, 0, cc))
            for t in range(1, 9):
                nc.vector.tensor_tensor(out=acblk(cc, 0), in0=acblk(cc, 0),
                                        in1=wblk(0, t, cc), op=ALU.add)
            for g, taps in ((1, [6, 7, 8]), (2, [0, 1, 2]),
                            (3, [2, 5, 8]), (4, [0, 3, 6])):
                nc.vector.tensor_copy(acblk(cc, g), wblk(0, taps[0], cc))
                for t in taps[1:]:
                    nc.vector.tensor_tensor(out=acblk(cc, g),
                                            in0=acblk(cc, g),
                                            in1=wblk(0, t, cc), op=ALU.add)
    gapH = [gsm.tile([128, BL], F32, tag=f"gapH{cc}", name=f"gapHs{cc}")
            for cc in range(NCC)]

    # ---- natural loads -----------------------------------------------------
    natI, natH, natS = {}, {}, {}     # stage-B tiles
    aI, aH = {}, {}                   # stage-A-only tiles (j=3..7)

    def load_nat(j, which):
        """one 3D DMA: [112, 7, 256] natural-layout tile"""
        if which == "in":
            src, p, tag = dram["inputs"], natb, "natb"
        elif which == "hid":
            src, p, tag = dram["hidden_prev"], natb, "natb"
        elif which == "ain":
            src, p, tag = dram["inputs"], nata, "nata"
        elif which == "ahid":
            src, p, tag = dram["hidden_prev"], nata, "nata"
        else:
            src, p, tag = dram["state_prev"], natsp, "natsp"
        t = p.tile([PG, NPG, 256], BF16, tag=tag, name=f"nat_{which}_{j}")
        q = nc.scalar if which == "hid" else nc.sync
        q.dma_start(t[:], src[j].rearrange("(g p) d -> p g d", p=PG))
        return t

    # ---- stage A: masked pixel sums off a natural tile ---------------------
    def emit_stage_a(j, nat, is_input):
        p9 = ps_gap.tile([9, 256], F32, tag="rawT")
        for pg in range(NPG):
            nc.tensor.matmul(p9[:], masks[pg][:],
                             nat[:, pg, :],
                             start=(pg == 0), stop=(pg == NPG - 1))
        rt = gtmp.tile([9, 256], F32, tag="rawT_sb", bufs=1)
        nc.vector.tensor_copy(rt[:], p9[:])
        for cc in range(NCC):
            cs = slice(cc * 128, (cc + 1) * 128)
            ptw = ps_tr.tile([128, PG], F32, tag="ptr")
            pt = ptw[:, 0:9]
            nc.tensor.transpose(pt, rt[:, cs], ident[0:9, 0:9])
            if is_input:
                r3 = _r3(raw[cc][:], BL)
                nc.vector.tensor_copy(
                    r3[:, :, j:j + 1],
                    pt.rearrange("c (n o) -> c n o", o=1))
                # fold the negative sign of mask groups 1-4 into raw
                nc.vector.tensor_scalar_mul(r3[:, 1:5, j:j + 1],
                                            r3[:, 1:5, j:j + 1], -1.0)
            else:
                nc.vector.tensor_copy(gapH[cc][:, j:j + 1], pt[:, 0:1])

    # ---- stage B: transposes into padded conv layout -----------------------
    xin, xhid, xsp = {}, {}, {}

    def alloc_xt(j, which):
        if which == "in":
            p, dst, tag = xt_in, xin, "xin"
        elif which == "hid":
            p, dst, tag = xt_hid, xhid, "xhid"
        else:
            p, dst, tag = xt_sp, xsp, "xsp"
        tiles = []
        for cc in range(NCC):
            xlen = PIX if which == "sp" else XTLEN
            xt = p.tile([128, xlen], BF16, tag=f"{tag}{cc}",
                        name=f"{tag}{cc}_{j}")
            if which != "sp":
                x3 = _r3(xt[:], PAD)
                nc.gpsimd.memset(x3[:, 0:1, :], 0.0)
                nc.gpsimd.memset(x3[:, PAD - 1:PAD, :], 0.0)
                nc.gpsimd.memset(x3[:, 1:PAD - 1, 0:1], 0.0)
                nc.gpsimd.memset(x3[:, 1:PAD - 1, PAD - 1:PAD], 0.0)
            tiles.append(xt)
        dst[j] = tiles
        return tiles

    def emit_transpose_group(nat, tiles, g7, padded):
        for cc in range(NCC):
            if padded:
                pt = ps_tr.tile([128, PG], BF16, tag="ptr")
                nc.tensor.transpose(pt[:],
                                    nat[:, g7, cc * 128:(cc + 1) * 128],
                                    identb[0:PG, 0:PG])
                dst = _r3(tiles[cc][:], PAD)[:, 1 + 4 * g7:1 + 4 * g7 + 4,
                                             1:29]
                nc.vector.tensor_copy(dst,
                                      pt[:].rearrange("c (r w) -> c r w", w=W))
            else:
                pt = ps_tr.tile([128, PG], BF16, tag="ptr")
                nc.tensor.transpose(pt[:],
                                    nat[:, g7, cc * 128:(cc + 1) * 128],
                                    identb[0:PG, 0:PG])
                dst = _r3(tiles[cc][:, g7 * PG:(g7 + 1) * PG], W)
                nc.vector.tensor_copy(dst,
                                      pt[:].rearrange("c (r w) -> c r w", w=W))

    # ---- conv window: 36 matmuls + DVE bias add + ACT tanh -----------------
    def emit_conv_window(j, wi, dc, r0=None, nr=WROWS):
        if r0 is None:
            r0 = wi * WROWS
        h0 = 1 + r0
        base = r0 * W
        wn = nr * W
        p = ps_conv.tile([128, WN], F32, tag="pconv", name="pconv")
        p3 = _r3(p[:, 0:wn], W)
        first = True
        for conv, xbuf in ((0, xin[j]), (1, xhid[j])):
            for t, (kh, kw) in enumerate(TAPS):
                dh, dwid = kh - 1, kw - 1
                for cc in range(NCC):
                    rhs = _r3(xbuf[cc][:], PAD)[
                        :, h0 + dh:h0 + dh + nr, 1 + dwid:1 + dwid + W]
                    last = (conv == 1 and t == 8 and cc == NCC - 1)
                    nc.tensor.matmul(
                        p3, wblk(conv, t, cc)[:, dc * 128:(dc + 1) * 128],
                        rhs, start=first, stop=last)
                    first = False
        pre = ew.tile([128, WN], F32, tag="pre", bufs=3, name="pre")
        nc.vector.tensor_tensor(out=pre[:, 0:wn], in0=p[:, 0:wn],
                                in1=gbias[dc][:, base:base + wn], op=ALU.add)
        gt = gtp.tile([128, WN], BF16, tag="gt", name=f"gt{j}_{wi}_{dc}")
        nc.scalar.activation(gt[:, 0:wn], pre[:, 0:wn], AF.Tanh)
        return gt

    # ---- elementwise + per-window store ------------------------------------
    gates = {}

    def emit_elementwise(j, wi, dc, gt, r0=None, nr=WROWS):
        if r0 is None:
            r0 = wi * WROWS
        base = r0 * W
        wn = nr * W
        t0 = j * H + r0

        def gw(gate):
            return gates[gate][dc][:, t0:t0 + nr].to_broadcast(
                [128, nr, W])

        sp3 = _r3(xsp[j][dc][:, base:base + wn], W)
        g3 = _r3(gt[:, 0:wn], W)
        s1 = ew.tile([128, WN], F32, tag="s1", name="s1")
        nc.gpsimd.tensor_tensor(out=_r3(s1[:, 0:wn], W), in0=sp3, in1=gw("f"),
                                op=ALU.mult)
        s2 = ew.tile([128, WN], F32, tag="s2", name="s2")
        nc.gpsimd.tensor_tensor(out=_r3(s2[:, 0:wn], W), in0=g3, in1=gw("i"),
                                op=ALU.mult)
        st = outb.tile([128, WN], F32, tag="st", name=f"st{j}_{wi}_{dc}")
        nc.gpsimd.tensor_tensor(out=_r3(st[:, 0:wn], W),
                                in0=_r3(s1[:, 0:wn], W),
                                in1=_r3(s2[:, 0:wn], W), op=ALU.add)
        th = ew.tile([128, WN], F32, tag="th", name="th")
        nc.scalar.activation(th[:, 0:wn], st[:, 0:wn], AF.Tanh)
        hd = outb.tile([128, WN], F32, tag="hd", name=f"hd{j}_{wi}_{dc}")
        nc.gpsimd.tensor_tensor(out=_r3(hd[:, 0:wn], W),
                                in0=_r3(th[:, 0:wn], W),
                                in1=gw("o"), op=ALU.mult)
        nc.sync.dma_start(dram["state"][j, dc][:, base:base + wn],
                          st[:, 0:wn])
        nc.scalar.dma_start(dram["hidden"][j, dc][:, base:base + wn],
                            hd[:, 0:wn])

    # ---- PE-side work queue (drained between conv windows) -----------------
    peq = deque()

    def drain(n):
        for _ in range(min(n, len(peq))):
            peq.popleft()()

    def queue_batch_transposes(j):
        ti = alloc_xt(j, "in")
        th_ = alloc_xt(j, "hid")
        groups = []
        for g7 in range(NPG):
            groups.append(lambda g7=g7, j=j, ti=ti: emit_transpose_group(
                natI[j], ti, g7, True))
            groups.append(lambda g7=g7, j=j, th_=th_: emit_transpose_group(
                natH[j], th_, g7, True))
        if j == 0:
            groups = groups[0::2] + groups[1::2]
        peq.extend(groups)

    def queue_sp_transposes(j):
        ts = alloc_xt(j, "sp")
        for g7 in range(NPG):
            peq.append(lambda g7=g7, j=j, ts=ts: emit_transpose_group(
                natS[j], ts, g7, False))

    def queue_stage_a(j):
        if j <= 2:
            peq.append(lambda j=j: emit_stage_a(j, natI[j], True))
            peq.append(lambda j=j: emit_stage_a(j, natH[j], False))
        else:
            peq.append(lambda j=j: emit_stage_a(j, aI[j], True))
            peq.append(lambda j=j: emit_stage_a(j, aH[j], False))

    # ---- gap combine + collective ------------------------------------------
    def emit_gap_combine_and_allgather():
        gap_ps = ps_gap.tile([8, 256], F32, tag="gapI")
        terms = []
        for cc in range(NCC):
            for g in range(5):
                terms.append((g, acblk(cc, g), cc))
            for g, t in ((5, 8), (6, 6), (7, 2), (8, 0)):
                terms.append((g, wblk(0, t, cc), cc))
        for i, (g, mov, cc) in enumerate(terms):
            nc.tensor.matmul(gap_ps[:], raw[cc][:, g * BL:(g + 1) * BL],
                             mov,
                             start=(i == 0), stop=(i == len(terms) - 1))
        gapI_sb = gsm.tile([8, 256], F32, tag="gapI_sb")
        nc.vector.tensor_copy(gapI_sb[:], gap_ps[:])
        nc.sync.dma_start(dram["cc_in"][0:8, :], gapI_sb[:, 0:128])
        nc.sync.dma_start(dram["cc_in"][8:16, :], gapI_sb[:, 128:256])
        for cc in range(NCC):
            pt = ps_gap.tile([8, 128], F32, tag="gapI")
            nc.tensor.transpose(pt[:], gapH[cc][:], ident[:])
            hs = gsm.tile([8, 128], F32, tag=f"gapH_sb{cc}",
                          name=f"gapHsb{cc}")
            nc.vector.tensor_copy(hs[:], pt[:])
            nc.sync.dma_start(dram["cc_in"][16 + 8 * cc:24 + 8 * cc, :], hs[:])
        nc.gpsimd.collective_compute(
            "AllGather", ALU.bypass, replica_groups=[list(range(N_CORES))],
            ins=[dram["cc_in"][:]], outs=[dram["cc_out"][:]])

    def emit_gather_and_gates():
        sel = [gsm.tile([128, 224], BF16, tag=f"sel{g4}", name=f"sel{g4}")
               for g4 in range(4)]
        for g4 in range(4):
            for hf in range(2):
                gtile = gtmp.tile([PG, 128], F32, tag="gath", name="gath")
                nc.gpsimd.indirect_dma_start(
                    out=gtile[:], out_offset=None, in_=dram["cc_out"][:],
                    in_offset=bass.IndirectOffsetOnAxis(
                        ap=idx_sb[g4][hf][:, :1], axis=0))
                pt = ps_tr.tile([128, PG], F32, tag="ptr", name="pt_gath")
                nc.tensor.transpose(pt[:], gtile[:],
                                    ident[0:PG, 0:PG])
                nc.vector.tensor_copy(sel[g4][:, hf * PG:(hf + 1) * PG],
                                      pt[:])
        for gate, wx, wh, bi in (("i", "wxi", "whi", "inputBias"),
                                 ("f", "wxf", "whf", "forgetBias"),
                                 ("o", "wxo", "who", "outputBias")):
            per_dc = []
            for dc in range(NDC):
                t1 = gtmp.tile([128, 224], F32, tag="gm1", bufs=1, name="gm1")
                nc.vector.tensor_scalar_mul(t1[:], sel[dc][:],
                                            vecs[wx][:, dc:dc + 1])
                t2 = gtmp.tile([128, 224], F32, tag="gm2", bufs=1, name="gm2")
                nc.vector.tensor_scalar_mul(t2[:], sel[2 + dc][:],
                                            vecs[wh][:, dc:dc + 1])
                nc.vector.tensor_tensor(out=t1[:], in0=t1[:], in1=t2[:],
                                        op=ALU.add)
                gtb = gsm.tile([128, 224], F32, tag=f"gate_{gate}{dc}",
                               name=f"gate_{gate}{dc}")
                nc.scalar.activation(gtb[:], t1[:], AF.Sigmoid,
                                     bias=vecs[bi][:, dc:dc + 1])
                per_dc.append(gtb)
            gates[gate] = per_dc

    # ======================= emission schedule ==============================
    # b0 stage-B loads + identity first on the sync queue; weights first on
    # the act queue. Stage-A-only loads are paced in the slot loop (their DMA
    # dispatch can block the sync queue on buffer-free waits, so nothing
    # deadline-critical may follow them in the same slot).
    natI[0] = load_nat(0, "in")
    natH[0] = load_nat(0, "hid")
    # identity built on-chip: iota(p - i) == 0 selects the diagonal
    ident = const.tile([128, 128], F32, tag="ident")
    nc.gpsimd.memset(ident[:], 1.0)
    nc.gpsimd.affine_select(ident[:], ident[:], [[-1, 128]],
                            ALU.is_equal, 0.0, base=0, channel_multiplier=1)
    identb = const.tile([128, 128], BF16, tag="identb")
    nc.vector.tensor_copy(identb[:], ident[:])
    load_weights()
    masks = []
    for pg in range(NPG):
        m = const.tile([PG, 9], BF16, tag=f"mask{pg}", name=f"mask{pg}")
        nc.sync.dma_start(m[:], dram["masks"][pg])
        masks.append(m)
    natI[1] = load_nat(1, "in")
    natH[1] = load_nat(1, "hid")
    gnat = wst.tile([PG, NPG, 256], F32, tag="gnat", bufs=1)
    nc.sync.dma_start(gnat[:],
                      dram["gb"].rearrange("(g p) d -> p g d", p=PG))
    load_idx_and_vecs()

    def emit_gbias_group(g7):
        for dc in range(NDC):
            pt = ps_tr.tile([128, PG], F32, tag="ptr")
            nc.tensor.transpose(
                pt[:], gnat[:, g7, dc * 128:(dc + 1) * 128],
                ident[0:PG, 0:PG])
            nc.vector.tensor_copy(gbias[dc][:, g7 * PG:(g7 + 1) * PG],
                                  pt[:])

    # prologue PE work, drained now: batch-0 transposes + stage A for b0 +
    # gbias transposes (the first conv window's bias-add reads gbias, so it
    # must be written before slot 0 in DVE program order).
    queue_batch_transposes(0)
    queue_stage_a(0)
    for g7 in range(NPG):
        peq.append(lambda g7=g7: emit_gbias_group(g7))
    drain(len(peq))
    queue_batch_transposes(1)
    queue_stage_a(1)

    gts = {}
    for k in range(BL):
        # just-in-time loads (stage-B two batches ahead; stage-A paced)
        if k + 2 < BL:
            natI[k + 2] = load_nat(k + 2, "in")
            natH[k + 2] = load_nat(k + 2, "hid")
        if k == 0:
            for j in (3, 4):
                aI[j] = load_nat(j, "ain")
                aH[j] = load_nat(j, "ahid")
        elif k == 2:
            for j in (5, 6, 7):
                aI[j] = load_nat(j, "ain")
                aH[j] = load_nat(j, "ahid")
            for j in range(0, 4):
                natS[j] = load_nat(j, "sp")
        elif k >= 3 and k + 1 < BL:
            natS[k + 1] = load_nat(k + 1, "sp")

        # queue PE side work for this slot (b1 already queued in prologue)
        if 1 <= k and k + 1 < BL:
            queue_batch_transposes(k + 1)
        if k == 0:
            queue_stage_a(2)
        elif k == 1:
            queue_stage_a(3)
            queue_stage_a(4)
        elif k == 2:
            queue_stage_a(5)
            queue_stage_a(6)
            queue_stage_a(7)
        if k >= 5 and k + 1 < BL:
            queue_sp_transposes(k + 1)

        nwin = NW * NDC
        for wi in range(NW):
            for dc in range(NDC):
                halves = (k == BL - 1 and wi == NW - 1 and dc == NDC - 1)
                if halves:
                    hw_ = WROWS // 2
                    for hh in range(2):
                        gt = emit_conv_window(k, wi, dc,
                                              r0=wi * WROWS + hh * hw_,
                                              nr=hw_)
                        drain(len(peq))
                        emit_elementwise(k, wi, dc, gt,
                                         r0=wi * WROWS + hh * hw_, nr=hw_)
                    continue
                gt = emit_conv_window(k, wi, dc)
                rem = len(peq)
                left = nwin - (wi * NDC + dc)
                drain(max(5, (rem + left - 1) // left))
                # ew is emitted after the drain so its sp-layout transposes
                # (front of the queue at k>=5) precede it in DVE order
                if k <= 4:
                    gts[(k, wi, dc)] = gt
                else:
                    emit_elementwise(k, wi, dc, gt)

        if k == 1:
            emit_a_combine()
        if k == 2:
            drain(len(peq))           # all stage A must be emitted
            emit_gap_combine_and_allgather()
        if k == 4:
            drain(len(peq))
            emit_gather_and_gates()
            for j in range(0, 4):
                queue_sp_transposes(j)
            drain(len(peq))
            for j in range(0, 2):
                for wi in range(NW):
                    for dc in range(NDC):
                        emit_elementwise(j, wi, dc, gts.pop((j, wi, dc)))
            queue_sp_transposes(4)
            drain(len(peq))
            for j in range(2, 5):
                for wi in range(NW):
                    for dc in range(NDC):
                        emit_elementwise(j, wi, dc, gts.pop((j, wi, dc)))
            queue_sp_transposes(5)
    drain(len(peq))

    ctx.close()


# ---------------------------------------------------------------------------
_NC_CACHE = None


def _get_nc():
    global _NC_CACHE
    if _NC_CACHE is None:
        nc = build_nc()
        _split_excess_sem_waits(nc)
        _NC_CACHE = nc
    return _NC_CACHE


def _gate_idx(core):
    idx = np.empty((4, 224, 1), np.int32)
    for j in range(BL):
        for hh in range(H):
            t = j * H + hh
            sel_b = (H * (BL * core + j) + hh) % B
            cp, bp = sel_b // BL, sel_b % BL
            for g in range(4):
                idx[g, t, 0] = cp * 32 + g * 8 + bp
    return idx


def _gap_masks():
    m = np.zeros((PIX, 9), np.float32)
    hw = np.arange(PIX)
    r, c = hw // W, hw % W
    m[:, 0] = 1.0
    m[r == 0, 1] = 1.0
    m[r == H - 1, 2] = 1.0
    m[c == 0, 3] = 1.0
    m[c == W - 1, 4] = 1.0
    m[(r == 0) & (c == 0), 5] = 1.0
    m[(r == 0) & (c == W - 1), 6] = 1.0
    m[(r == H - 1) & (c == 0), 7] = 1.0
    m[(r == H - 1) & (c == W - 1), 8] = 1.0
    return m.reshape(NPG, PG, 9)


def _make_in_maps(inputs):
    f32 = np.float32
    import ml_dtypes
    wpk = np.empty((128, 2 * 9 * NCC * 256), dtype=ml_dtypes.bfloat16)
    for conv, w in ((0, np.asarray(inputs["wconvInput"], dtype=f32)),
                    (1, np.asarray(inputs["wconvHidden"], dtype=f32))):
        for t, (kh, kw) in enumerate(TAPS):
            for cc in range(NCC):
                off = ((conv * 9 + t) * NCC + cc) * 256
                wpk[:, off:off + 256] = w[kh, kw,
                                          cc * 128:(cc + 1) * 128, :]
    shared = {
        "wconv_bf16": wpk,
        "gateBias": np.ascontiguousarray(inputs["gateBias"],
                                         dtype=f32).reshape(PIX, D),
        "gap_masks": _gap_masks().astype(ml_dtypes.bfloat16),
    }
    for nm in ("wxi", "whi", "inputBias", "wxf", "whf", "forgetBias",
               "wxo", "who", "outputBias"):
        shared[nm] = np.ascontiguousarray(inputs[nm], dtype=f32).reshape(D, 1)

    bf16 = ml_dtypes.bfloat16
    xin = np.ascontiguousarray(inputs["inputs"], dtype=f32).reshape(
        B, PIX, CIN).astype(bf16)
    xsp = np.ascontiguousarray(inputs["state_prev"], dtype=f32).reshape(
        B, PIX, D).astype(bf16)
    xhp = np.ascontiguousarray(inputs["hidden_prev"], dtype=f32).reshape(
        B, PIX, D).astype(bf16)

    in_maps = []
    for k in range(N_CORES):
        sl = slice(k * BL, (k + 1) * BL)
        m = dict(shared)
        m["inputs"] = xin[sl]
        m["state_prev"] = xsp[sl]
        m["hidden_prev"] = xhp[sl]
        m["gate_idx"] = _gate_idx(k)
        in_maps.append(m)
    return in_maps


def kernel(**inputs):
    nc = _get_nc()
    in_maps = _make_in_maps(inputs)
    res = run_bass_kernel_spmd(nc, in_maps, core_ids=list(range(N_CORES)))

    def unshard(name):
        # per-core outputs are [BL, NDC, 128, PIX] (channel-major); restore NHWC
        full = np.concatenate([res.results[k][name] for k in range(N_CORES)],
                              axis=0)
        return np.ascontiguousarray(full.transpose(0, 3, 1, 2)).reshape(
            B, H, W, D)

    return unshard("hidden"), unshard("state")
